# revision 1
# baseline (speedup 1.0000x reference)
"""Trainium2 Bass kernel for a binarized 4-layer MLP (dense_mlp).

Net (per reference):
  h = sign(x) @ sign(w1).T + b1 ; h = clip(bn1(h), -1, 1)
  h = sign(h) @ sign(w2).T + b2 ; h = clip(bn2(h), -1, 1)
  h = sign(h) @ sign(w3).T + b3 ; h = clip(bn3(h), -1, 1)
  logits = h @ w4.T + b4 ; out = log_softmax(logits)   # 2 classes

Strategy: pure data parallel over 8 cores (batch 131072 -> 8 x 16384).
Host prep: x transposed to feature-major [81, B] and sharded by columns;
sign/BN/bias/log-softmax algebra folded into device weights + thresholds.

On-device per core (feature-major activations, exact bf16 binarized matmuls):
  - u0 = [x > 0] in {0,1} bf16 (GpSimd compare, SBUF->SBUF)
  - sign activations propagate as {0,1} "u-form" (DVE is_gt) or +-1 "s-form"
    (ACT Sign); the 2x / rowsum corrections fold into the next layer's
    weights (+-2, exact in bf16) and per-feature thresholds (host).
  - L3: s3 (BN scale) folds into w3; clip computed as t = min(p+e3, 1) on
    DVE, then r2 = Relu(t+1) = h3+1 on ACT (the -1 folds into the head).
  - head: d = r2 @ dw accumulated in a [8,512] PSUM tile (one row per
    512-col chunk), SBUF->SBUF DMA re-spreads to batch-major [32,128],
    out0 = ln(sigmoid(-(d+db'))), out1 = ln(sigmoid(d+db')).
"""

import os
import sys

import numpy as np

for _p in ("/opt/trn_rl_repo", "/root/.axon_site/_ro/trn_rl_repo"):
    if os.path.isdir(_p) and _p not in sys.path:
        sys.path.insert(0, _p)

import ml_dtypes  # noqa: E402

BF16 = ml_dtypes.bfloat16
FP8 = ml_dtypes.float8_e4m3

# Problem constants (hardcoded per contract)
B_FULL = 131072
N_CORES = 8
NB = B_FULL // N_CORES  # 16384 rows per core
IN = 81
H = 384
EPS = 1e-5
P = 128
WCH = 1024          # free-dim per elementwise tile (2 PSUM banks)
G_NCH = 8           # 512-col chunks per super-chunk
G_COLS = G_NCH * 512  # 4096
N_GROUPS = NB // G_COLS
WPG = G_COLS // WCH
RPT = G_COLS // P  # tail rows per group

_CACHE = {}


def _build_program():
    import concourse.bacc as bacc
    import concourse.bass as bass  # noqa: F401
    import concourse.tile as tile
    from concourse import mybir

    f32 = mybir.dt.float32
    bf16 = mybir.dt.bfloat16
    fp8 = mybir.dt.float8e4
    DR = mybir.MatmulPerfMode.DoubleRow
    AF = mybir.ActivationFunctionType
    ALU = mybir.AluOpType

    nc = bacc.Bacc("TRN2", target_bir_lowering=False, debug=False)

    xt_d = nc.dram_tensor("xt", [IN, NB], f32, kind="ExternalInput").ap()
    w1t_d = nc.dram_tensor("w1t", [IN, H], bf16, kind="ExternalInput").ap()
    w2t_d = nc.dram_tensor("w2t", [P, 1536], fp8, kind="ExternalInput").ap()
    w3t_d = nc.dram_tensor("w3t", [P, 1536], fp8, kind="ExternalInput").ap()
    dwt_d = nc.dram_tensor("dwt", [P, 3], bf16, kind="ExternalInput").ap()
    aux_d = nc.dram_tensor("aux", [P, 14], f32, kind="ExternalInput").ap()
    out_d = nc.dram_tensor("out", [NB, 2], f32, kind="ExternalOutput").ap()

    with tile.TileContext(nc) as tc:
        with (
            tc.tile_pool(name="consts", bufs=1) as cpool,
            tc.tile_pool(name="xin", bufs=2) as xpool,
            tc.tile_pool(name="u0", bufs=4) as u0pool,
            tc.tile_pool(name="acts", bufs=8) as apool,
            tc.tile_pool(name="tclip", bufs=4) as tpool,
            tc.tile_pool(name="h3", bufs=14) as h3pool,
            tc.tile_pool(name="dsb", bufs=1) as dsbpool,
            tc.tile_pool(name="fin", bufs=2) as fpool,
            tc.tile_pool(name="mm", bufs=3, space="PSUM") as pspool,
            tc.tile_pool(name="mmd", bufs=2, space="PSUM") as psdpool,
            tc.tile_pool(name="dram", bufs=1, space="DRAM") as dpool,
        ):
            dscr = dpool.tile([NB], f32)
            # ---- constants ----
            w1s = cpool.tile([IN, H], bf16)
            nc.sync.dma_start(w1s[:], w1t_d[:])
            w2s = cpool.tile([P, 1536], fp8)
            nc.sync.dma_start(w2s[:], w2t_d[:])
            w3s = cpool.tile([P, 1536], fp8)
            nc.sync.dma_start(w3s[:], w3t_d[:])
            w2r = w2s.rearrange("p (s i c) -> p s i c", i=2, c=P)
            w3r = w3s.rearrange("p (s i c) -> p s i c", i=2, c=P)
            dws = cpool.tile([P, 3], bf16)
            nc.sync.dma_start(dws[:], dwt_d[:])
            aux = cpool.tile([P, 14], f32)
            nc.sync.dma_start(aux[:], aux_d[:])

            for g in range(N_GROUPS):
                col0 = g * G_COLS
                # ---- stage X: load xT slices, compare on DVE ----
                u0t = []
                for w in range(WPG // 2):
                    xf = xpool.tile([IN, 2 * WCH], f32)
                    nc.sync.dma_start(
                        xf[:],
                        xt_d[:, col0 + w * 2 * WCH : col0 + (w + 1) * 2 * WCH],
                    )
                    u0 = u0pool.tile([IN, 2 * WCH], bf16)
                    nc.vector.tensor_scalar(u0[:], xf[:], 0.0, None, ALU.is_gt)
                    u0t.append(u0)

                # ---- L1 ----
                u1 = []
                for w in range(WPG):
                    ua = apool.tile([P, 3, WCH], fp8, tag="u1")
                    u1.append(ua)
                for m in range(3):
                    for w in range(WPG):
                        ps = pspool.tile([P, WCH], f32, tag="ps")
                        for h in range(2):
                            c0 = w * WCH + h * 512
                            nc.tensor.matmul(
                                ps[:, h * 512 : (h + 1) * 512],
                                w1s[:, m * P : (m + 1) * P],
                                u0t[c0 // (2 * WCH)][
                                    :, c0 % (2 * WCH) : c0 % (2 * WCH) + 512
                                ],
                                start=True,
                                stop=True,
                            )
                        if m < 2:
                            nc.scalar.activation(
                                u1[w][:, m, :], ps[:], AF.Sign,
                                bias=aux[:, m : m + 1], scale=1.0
                            )
                        else:
                            nc.vector.tensor_scalar(
                                u1[w][:, m, :], ps[:], aux[:, 2:3], None,
                                ALU.is_gt
                            )

                # ---- L2 (fp8 DoubleRow, K=512 padded) ----
                u2 = []
                for w in range(WPG):
                    ua = apool.tile([P, 3, WCH], fp8, tag="u2")
                    u2.append(ua)
                for m in range(3):
                    for wp in range(WPG // 2):
                        pss = [
                            pspool.tile([P, WCH], f32, tag="ps", name=f"ps2_{g}_{m}_{wp}_{wi}")
                            for wi in range(2)
                        ]
                        # kh-outer, 4 MMs per weight load
                        for kh in range(2):
                            for wi in range(2):
                                w = wp * 2 + wi
                                for h in range(2):
                                    nc.tensor.matmul(
                                        pss[wi][:, h * 512 : (h + 1) * 512],
                                        w2r[:, kh * 3 + m, :, :],
                                        u1[w][:, kh : kh + 2,
                                              h * 512 : (h + 1) * 512],
                                        start=(kh == 0),
                                        stop=(kh == 1),
                                        perf_mode=DR,
                                    )
                        for wi in range(2):
                            w = wp * 2 + wi
                            if m < 2:
                                nc.scalar.activation(
                                    u2[w][:, m, :], pss[wi][:], AF.Sign,
                                    bias=aux[:, 3 + m : 4 + m], scale=1.0
                                )
                            else:
                                nc.vector.tensor_scalar(
                                    u2[w][:, m, :], pss[wi][:], aux[:, 5:6],
                                    None, ALU.is_gt
                                )

                # ---- L3 (fp8 DoubleRow) + clip ----
                h3 = [[None] * WPG for _ in range(3)]
                for m in range(3):
                    for wp in range(WPG // 2):
                        pss = [
                            pspool.tile([P, WCH], f32, tag="ps", name=f"ps3_{g}_{m}_{wp}_{wi}")
                            for wi in range(2)
                        ]
                        for kh in range(2):
                            for wi in range(2):
                                w = wp * 2 + wi
                                for h in range(2):
                                    nc.tensor.matmul(
                                        pss[wi][:, h * 512 : (h + 1) * 512],
                                        w3r[:, kh * 3 + m, :, :],
                                        u2[w][:, kh : kh + 2,
                                              h * 512 : (h + 1) * 512],
                                        start=(kh == 0),
                                        stop=(kh == 1),
                                        perf_mode=DR,
                                    )
                        for wi in range(2):
                            w = wp * 2 + wi
                            # y3 = s3*p3 + e3 (fp32)
                            t = tpool.tile([P, WCH], f32, tag="t3", name=f"t3_{g}_{m}_{wp}_{wi}")
                            if m < 2:
                                nc.scalar.activation(
                                    t[:], pss[wi][:], AF.Identity,
                                    bias=aux[:, 6 + m : 7 + m],
                                    scale=aux[:, 9 + m : 10 + m],
                                )
                            else:
                                nc.vector.tensor_scalar(
                                    t[:], pss[wi][:],
                                    aux[:, 9 + m : 10 + m], aux[:, 6 + m : 7 + m],
                                    ALU.mult, ALU.add,
                                )
                            # h3 = clip(y3) -> bf16 (DVE)
                            h3c = h3pool.tile([P, WCH], bf16, tag="h3", name=f"h3_{g}_{m}_{wp}_{wi}")
                            nc.vector.tensor_scalar(
                                h3c[:], t[:], 1.0, -1.0, ALU.min, ALU.max
                            )
                            h3[m][w] = h3c

                # ---- head: d per 512-chunk in [1,512] PSUM tiles ----
                dsb = dsbpool.tile([1, G_COLS], f32)
                for r in range(G_NCH):
                    w, h = r // 2, r % 2
                    psd = psdpool.tile([1, 512], f32, tag="psd")
                    for k in range(3):
                        nc.tensor.matmul(
                            psd[:],
                            dws[:, k : k + 1],
                            h3[k][w][:, h * 512 : (h + 1) * 512],
                            start=(k == 0),
                            stop=(k == 2),
                        )
                    dst = dsb[0:1, r * 512 : (r + 1) * 512]
                    if r % 2 == 0:
                        nc.vector.tensor_copy(dst, psd[:])
                    else:
                        nc.scalar.copy(dst, psd[:])

                # re-spread to batch-major [32, 128] via DRAM bounce
                # (direct SBUF->SBUF partition-spread DMA scrambles on HW)
                dsl = dscr[g * G_COLS : (g + 1) * G_COLS]
                nc.sync.dma_start(
                    dsl.rearrange("(one f) -> one f", one=1), dsb[:]
                )
                d2 = fpool.tile([RPT, P], f32, tag="d2")
                nc.sync.dma_start(d2[:], dsl.rearrange("(p j) -> p j", j=P))
                sneg = fpool.tile([RPT, P], f32, tag="sneg")
                nc.scalar.activation(
                    sneg[:], d2[:], AF.Sigmoid, bias=aux[0:RPT, 13:14], scale=-1.0
                )
                spos = fpool.tile([RPT, P], f32, tag="spos")
                nc.scalar.activation(
                    spos[:], d2[:], AF.Sigmoid, bias=aux[0:RPT, 12:13], scale=1.0
                )
                out_t = fpool.tile([RPT, 2 * P], f32, tag="outt")
                ov = out_t.rearrange("p (j c) -> p j c", c=2)
                nc.scalar.activation(ov[:, :, 0], sneg[:], AF.Ln)
                nc.scalar.activation(ov[:, :, 1], spos[:], AF.Ln)
                nc.sync.dma_start(
                    out_d[g * G_COLS : (g + 1) * G_COLS, :].rearrange(
                        "(p j) c -> p (j c)", j=P
                    ),
                    out_t[:],
                )

    nc.compile()
    return nc


def _get_program():
    if "nc" not in _CACHE:
        _CACHE["nc"] = _build_program()
    return _CACHE["nc"]


def _prep_consts(w1, b1, w2, b2, w3, b3, w4, b4,
                 g1, be1, m1, v1, g2, be2, m2, v2, g3, be3, m3, v3):
    """Host-side folding. Returns dict of device const arrays."""
    f8 = np.float64
    w1 = np.asarray(w1, f8); w2 = np.asarray(w2, f8); w3 = np.asarray(w3, f8)
    w4 = np.asarray(w4, f8)
    b1 = np.asarray(b1, f8); b2 = np.asarray(b2, f8); b3 = np.asarray(b3, f8)
    b4 = np.asarray(b4, f8)

    def fold(g, be, m, v, b):
        s = np.asarray(g, f8) / np.sqrt(np.asarray(v, f8) + EPS)
        c = s * (b - np.asarray(m, f8)) + np.asarray(be, f8)
        return s, c

    s1, c1 = fold(g1, be1, m1, v1, b1)
    s2, c2 = fold(g2, be2, m2, v2, b2)
    s3, c3 = fold(g3, be3, m3, v3, b3)

    W1s = np.sign(w1)  # [384, 81]
    W2s = np.sign(w2)  # [384, 384]
    W3s = np.sign(w3)

    # L1: all input features (u0) are u-form -> weights x2
    w1t = (2.0 * W1s).T.astype(BF16)  # [81, 384]

    # L2/L3 inputs: m0/m1 chunks (f<256) s-form (+-1), m2 u-form (x2)
    multf = np.where(np.arange(H) < 2 * P, 1.0, 2.0)
    W2eff = W2s * multf[None, :]
    W3eff = W3s * multf[None, :]

    def pack_lhsT_dr(Weff):
        # DoubleRow packing with overlapping rhs windows: kh=0 reads
        # activation planes (0,1) = features 0..255; kh=1 reads planes
        # (1,2) = features 128..383 with ZERO weights on the repeated
        # plane 1 (i=0), so no pad plane / memset is needed.
        t = np.zeros((P, 6, 2, P), dtype=f8)
        for m in range(3):
            for i in range(2):  # kh=0: features i*128..
                blk = Weff[m * P : (m + 1) * P, i * P : (i + 1) * P]
                t[:, m, i, :] = blk.T
            # kh=1: i=0 stays zero; i=1 = features 256..383
            blk = Weff[m * P : (m + 1) * P, 2 * P : 3 * P]
            t[:, 3 + m, 1, :] = blk.T
        return t.reshape(P, 1536).astype(FP8)

    w2t = pack_lhsT_dr(W2eff)
    w3t = pack_lhsT_dr(W3eff)

    # thresholds: u = [p > phi];  s-form ACT: sign(p - phi)
    phi1 = W1s.sum(axis=1) - c1 / s1
    phi2 = W2s[:, 2 * P :].sum(axis=1) - c2 / s2
    # L3: y3 = s3*p3 + e3 with exact +-1/+-2 weights;
    # correction subtracts s3 * sum_{u-form f} W3s[m,f]
    e3 = c3 - s3 * W3s[:, 2 * P :].sum(axis=1)

    dw = w4[1] - w4[0]
    db = b4[1] - b4[0]
    dbp = db  # all h3 chunks stored in clip-form

    dwt = np.zeros((P, 3), dtype=f8)
    for k in range(3):
        dwt[:, k] = dw[k * P : (k + 1) * P]
    dwt = dwt.astype(BF16)

    aux = np.zeros((P, 14), dtype=f8)
    aux[:, 0] = -phi1[0:P]
    aux[:, 1] = -phi1[P : 2 * P]
    aux[:, 2] = phi1[2 * P : 3 * P]
    aux[:, 3] = -phi2[0:P]
    aux[:, 4] = -phi2[P : 2 * P]
    aux[:, 5] = phi2[2 * P : 3 * P]
    for m in range(3):
        aux[:, 6 + m] = e3[m * P : (m + 1) * P]
        aux[:, 9 + m] = s3[m * P : (m + 1) * P]
    aux[:, 12] = dbp
    aux[:, 13] = -dbp
    aux = aux.astype(np.float32)

    return {"w1t": w1t, "w2t": w2t, "w3t": w3t, "dwt": dwt, "aux": aux}


def _make_in_maps(inputs):
    x = np.asarray(inputs["x"], np.float32)
    xt = np.ascontiguousarray(x.T)  # [81, 131072] feature-major
    consts = _prep_consts(
        inputs["w1"], inputs["b1"], inputs["w2"], inputs["b2"],
        inputs["w3"], inputs["b3"], inputs["w4"], inputs["b4"],
        inputs["g1"], inputs["be1"], inputs["m1"], inputs["v1"],
        inputs["g2"], inputs["be2"], inputs["m2"], inputs["v2"],
        inputs["g3"], inputs["be3"], inputs["m3"], inputs["v3"],
    )
    in_maps = []
    for i in range(N_CORES):
        m = {"xt": np.ascontiguousarray(xt[:, i * NB : (i + 1) * NB])}
        m.update(consts)
        in_maps.append(m)
    return in_maps


def kernel(**inputs):
    from concourse.bass_utils import run_bass_kernel_spmd

    nc = _get_program()
    in_maps = _make_in_maps(inputs)
    res = run_bass_kernel_spmd(nc, in_maps, list(range(N_CORES)))
    out = np.concatenate([res.results[i]["out"] for i in range(N_CORES)], axis=0)
    return out.astype(np.float32)

